# revision 2
# baseline (speedup 1.0000x reference)
"""AttentionPooling Trainium2 kernel v2: 8-core data-parallel over batch.

Math (validated in check_math_v2.py, rel err ~4e-3 vs reference):
 - Cross-attention softmaxes over one key -> collapses to c = x @ Wc.T,
   h1[b,l] = c[b] + (latents[l] + bc).
 - Self-attention over L=4, H=8, dh=48; 1/sqrt(dh) folded into Wq.
 - softmax exp via 3rd-order Taylor (|s| <= ~0.95) on Pool/DVE.
 - gelu -> Gelu_apprx_tanh; sigmoid(z) = 0.5*(1+tanh(z/2)); with Sqrt and
   Identity this costs 2 ACT table loads per macro (each macro's gate is
   deferred so its tanh lands adjacent to the next macro's gelu block).
 - Post-pool LN3 affine folded into gate weights; the sigmoid's 0.5 folded
   into the pooling matrix (0.125 entries), gate weights doubled.

Engine plan (GPSIMD/Pool cannot touch PSUM on hw!):
 - PE: all matmuls + transposes + rank-1 bias rows.
 - DVE: bn stats, LN scalar chains, attention mults (PSUM reads), reduces,
   residual adds, final gating multiply, transpose copies (some).
 - ACT: y1/y2 norm-applies (fused scale/bias Identity from PSUM), k/v
   PSUM->SBUF escapes, gelu, sqrt, tanh, some transpose copies.
 - Pool: softmax Taylor/normalize, o pair-adds, y3 apply (SBUF only).
"""

from contextlib import ExitStack

import numpy as np
import ml_dtypes

import concourse.bass as bass
import concourse.bacc as bacc_mod
import concourse.tile as tile
from concourse import mybir
from concourse.bass_utils import run_bass_kernel_spmd

D, H, L, B, NCORES = 384, 8, 4, 32768, 8
DH = D // H
BC = B // NCORES
P = 128
NMAC = BC // P
NSUB = 4
EPS = 1e-5

BF16 = ml_dtypes.bfloat16
F8 = ml_dtypes.float8_e4m3
f32 = mybir.dt.float32
bf16 = mybir.dt.bfloat16
f8 = mybir.dt.float8e4
AL = mybir.AluOpType
AF = mybir.ActivationFunctionType
AX = mybir.AxisListType
DR = mybir.MatmulPerfMode.DoubleRow


def _host_consts(inp):
    wq, wk, wv = np.split(inp["ca_w_in"], 3, axis=0)
    _, _, bv = np.split(inp["ca_b_in"], 3)
    Wc = inp["ca_w_out"] @ wv
    bc = inp["ca_w_out"] @ bv + inp["ca_b_out"]
    latb = inp["latents"][0] + bc[None, :]

    WsaT = inp["sa_w_in"].T.copy()
    WsaT[:, :D] *= 1.0 / np.sqrt(DH)
    bqkv = inp["sa_b_in"].copy()
    bqkv[:D] *= 1.0 / np.sqrt(DH)

    g3, b3 = inp["n3_g"], inp["n3_b"]
    WgT = (inp["gate_w"] * g3[None, :]).T * 2.0
    bg = inp["gate_b"] + b3 @ inp["gate_w"].T

    def chunkT(wT, nk):
        n = wT.shape[1]
        return np.ascontiguousarray(wT.reshape(nk, P, n).transpose(1, 0, 2))

    c = {}
    c["wc"] = chunkT(Wc.T, 3)
    c["wsa"] = chunkT(WsaT, 3)
    c["wso"] = chunkT(inp["sa_w_out"].T, 3)
    c["wg"] = chunkT(WgT, 3)
    c["latb"] = latb

    pidx = np.arange(P)
    Eall = np.zeros((P, NSUB, P), np.float32)
    for s in range(NSUB):
        Eall[32 * s + pidx // L, s, pidx] = 1.0
    c["emat"] = Eall
    oneL = np.zeros((L, P), np.float32)
    oneL[pidx % L, pidx] = 1.0
    c["onel"] = oneL
    Bl = np.zeros((P, L, P), np.float32)
    for lp in range(L):
        Bl[(pidx // L) * L + lp, lp, pidx] = 1.0
    c["bl"] = Bl
    pm = np.zeros((P, 32), np.float32)
    pm[pidx, pidx // L] = 0.125
    c["pool"] = pm
    c["ident"] = np.eye(P, dtype=np.float32)

    c["ones1"] = np.ones((1, P), np.float32)
    c["bqkv"] = bqkv[None, :]                              # [1, 1152]
    c["bso"] = inp["sa_b_out"][None, :]
    c["b2row"] = inp["ffn_b2"][None, :]
    c["bgrow"] = bg[None, :]
    consts = {k: v.astype(BF16) for k, v in c.items()}
    # fp8 DoubleRow-packed FFN weights: lhsT [K=128, pair, two, N]
    W1p = np.zeros((4 * P, 4 * D), np.float32)
    W1p[:D] = inp["ffn_w1"].T
    consts["w1dr"] = np.ascontiguousarray(
        W1p.reshape(2, 2, P, 4 * D).transpose(2, 0, 1, 3)).astype(F8)
    consts["w2dr"] = np.ascontiguousarray(
        inp["ffn_w2"].T.reshape(6, 2, P, D).transpose(2, 0, 1, 3)).astype(F8)
    consts["b1col"] = np.ascontiguousarray(
        inp["ffn_b1"].reshape(12, P).T).astype(np.float32)
    for nm in ("n1_g", "n1_b", "n2_g", "n2_b"):
        consts[nm] = np.broadcast_to(
            inp[nm][None, :], (P, D)).astype(BF16).copy()
    return consts


def _fancy(apbase, free_dims, extra_elem_offset=0):
    return bass.AP(
        tensor=apbase.tensor,
        offset=apbase.offset + extra_elem_offset,
        ap=[apbase.ap[0]] + [list(d) for d in free_dims],
    )


CONSTS_META = {
    "wc": ([P, 3, D], bf16), "wsa": ([P, 3, 3 * D], bf16),
    "wso": ([P, 3, D], bf16), "w1dr": ([P, 2, 2, 4 * D], f8),
    "w2dr": ([P, 6, 2, D], f8), "wg": ([P, 3, D], bf16),
    "latb": ([L, D], bf16), "emat": ([P, NSUB, P], bf16),
    "onel": ([L, P], bf16), "bl": ([P, L, P], bf16),
    "pool": ([P, 32], bf16), "ident": ([P, P], bf16),
    "ones1": ([1, P], bf16), "bqkv": ([1, 3 * D], bf16),
    "bso": ([1, D], bf16), "b2row": ([1, D], bf16), "bgrow": ([1, D], bf16),
    "b1col": ([P, 12], f32),
    "n1_g": ([P, D], bf16), "n1_b": ([P, D], bf16),
    "n2_g": ([P, D], bf16), "n2_b": ([P, D], bf16),
}


def build_program(repeat=1, aff1=False, aff2=False):
    nc = bacc_mod.Bacc("TRN2", target_bir_lowering=False, debug=False,
                       num_devices=NCORES)
    x_d = nc.declare_dram_parameter("x", [BC, D], f32, isOutput=False)
    cd = {k: nc.declare_dram_parameter(k, shp, dt, isOutput=False)
          for k, (shp, dt) in CONSTS_META.items()}
    out_d = nc.declare_dram_parameter("out", [BC, D], f32, isOutput=True)

    with tile.TileContext(nc) as tc, ExitStack() as ctx:
        consts = ctx.enter_context(tc.tile_pool(name="consts", bufs=1))
        io = ctx.enter_context(tc.tile_pool(name="io", bufs=3))
        act = ctx.enter_context(tc.tile_pool(name="act", bufs=2))
        stat = ctx.enter_context(tc.tile_pool(name="stat", bufs=4))
        ps = ctx.enter_context(tc.tile_pool(name="ps", bufs=3, space="PSUM"))

        cs = {}
        for k, (shp, dt) in CONSTS_META.items():
            cs[k] = consts.tile(shp, dt, name=f"c_{k}", tag=f"c_{k}")
            nc.sync.dma_start(out=cs[k][:], in_=cd[k][:])
        eps_t = consts.tile([P, 1], f32, tag="eps")
        nc.vector.memset(eps_t[:, :], EPS)
        half_t = consts.tile([P, 1], f32, tag="half")
        nc.vector.memset(half_t[:, :], 0.5)
        sixth_t = consts.tile([P, 1], f32, tag="sixth")
        nc.vector.memset(sixth_t[:, :], 1.0 / 6.0)
        one_t = consts.tile([P, 1], f32, tag="one")
        nc.vector.memset(one_t[:, :], 1.0)
        identf = consts.tile([P, P], f32, tag="identf")
        nc.vector.tensor_copy(out=identf[:], in_=cs["ident"][:])

        GELU = AF.Gelu_apprx_tanh

        def transp3(src_fn, dst_view, dtype=bf16, copy_eng="act"):
            """3 PE transposes into one PSUM tile; single fused copy out."""
            idt = cs["ident"] if dtype == bf16 else identf
            tp3 = ps.tile([P, 3 * P], dtype, tag="mm", name="tp3")
            for j in range(3):
                nc.tensor.transpose(tp3[:, j * P:(j + 1) * P], src_fn(j), idt[:])
            if copy_eng == "act":
                nc.scalar.activation(out=dst_view, in_=tp3[:, :],
                                     func=AF.Identity)
            else:
                nc.vector.tensor_copy(out=dst_view, in_=tp3[:, :])

        def ln_sub(src_ap, tag):
            """Per-subtile LN stats -> (rstd[128,1], nmr[128,1])."""
            st6 = stat.tile([P, 6], f32, tag="st6")
            nc.vector.bn_stats(out=st6[:, :], in_=src_ap)
            mv = stat.tile([P, 2], f32, tag=f"mv_{tag}")
            nc.vector.bn_aggr(out=mv[:, :], in_=st6[:, :])
            sig = stat.tile([P, 1], f32, tag=f"sig_{tag}")
            nc.scalar.activation(out=sig[:, :], in_=mv[:, 1:2], func=AF.Sqrt,
                                 bias=eps_t[:, 0:1], scale=1.0)
            rstd = stat.tile([P, 1], f32, tag=f"rstd_{tag}")
            nc.vector.reciprocal(out=rstd[:, :], in_=sig[:, :])
            nmr = stat.tile([P, 1], f32, tag=f"nmr_{tag}")
            nc.vector.scalar_tensor_tensor(
                out=nmr[:, :], in0=mv[:, 0:1], scalar=-1.0, in1=rstd[:, :],
                op0=AL.mult, op1=AL.mult)
            return rstd, nmr

        def emit_gate(pooled_sb):
            pT = act.tile([P, 3, P], bf16, tag="pT")
            transp3(lambda j: pooled_sb[:, j * P:(j + 1) * P],
                    _fancy(pT[:, 0, 0:1], [[1, 3 * P]]), copy_eng="act")
            gps = ps.tile([P, D], f32, tag="mm", name="gps")
            for k in range(3):
                nc.tensor.matmul(gps[:, :], pT[:, k, :], cs["wg"][:, k, :],
                                 start=(k == 0), stop=False)
            nc.tensor.matmul(gps[:, :], cs["ones1"][:, :], cs["bgrow"][:, :],
                             start=False, stop=True)
            th = act.tile([P, D], bf16, tag="th")
            nc.scalar.activation(out=th[:], in_=gps[:, :], func=AF.Tanh,
                                 scale=0.5)
            outf = io.tile([P, D], f32, tag="outf")
            nc.vector.scalar_tensor_tensor(
                out=outf[:], in0=th[:], scalar=1.0, in1=pooled_sb[:],
                op0=AL.add, op1=AL.mult)
            return outf

        rep_ctx = tc.For_i(0, repeat, 1) if repeat > 1 else None
        if rep_ctx is not None:
            ctx.enter_context(rep_ctx)

        pending = None
        for m in range(NMAC):
            # ---------- load + c ----------
            xt = io.tile([P, D], f32, tag="xin")
            nc.sync.dma_start(out=xt[:], in_=x_d[m * P:(m + 1) * P, :])
            xT = act.tile([P, 3, P], bf16, tag="xT")
            transp3(lambda j: xt[:, j * P:(j + 1) * P],
                    _fancy(xT[:, 0, 0:1], [[1, 3 * P]]), dtype=f32,
                    copy_eng="vec")
            cps = ps.tile([P, D], f32, tag="mm", name="cps")
            for k in range(3):
                nc.tensor.matmul(cps[:, :], xT[:, k, :], cs["wc"][:, k, :],
                                 start=(k == 0), stop=(k == 2))
            c_sb = act.tile([P, D], bf16, tag="c_sb")
            nc.scalar.activation(out=c_sb[:], in_=cps[:, :], func=AF.Identity)

            # ---------- per-subtile: h1/LN1/y1 then attention ----------
            y1 = act.tile([P, NSUB, D], bf16, tag="y1")
            r2 = act.tile([P, NSUB, D], bf16, tag="r2")
            y2 = act.tile([P, NSUB, D], bf16, tag="y2")
            for s in range(NSUB):
                h1ps = ps.tile([P, D], f32, tag="mm", name="h1ps")
                nc.tensor.matmul(h1ps[:, :], cs["emat"][:, s, :], c_sb[:],
                                 start=True, stop=False)
                nc.tensor.matmul(h1ps[:, :], cs["onel"][:, :], cs["latb"][:, :],
                                 start=False, stop=True)
                rstd1, nmr1 = ln_sub(h1ps[:, :], "ln1")
                nc.scalar.activation(out=y1[:, s, :], in_=h1ps[:, :],
                                     func=AF.Identity,
                                     bias=nmr1[:, 0:1], scale=rstd1[:, 0:1])
                if aff1:
                    nc.gpsimd.tensor_tensor(out=y1[:, s, :], in0=y1[:, s, :],
                                            in1=cs["n1_g"][:], op=AL.mult)
                    nc.gpsimd.tensor_add(y1[:, s, :], y1[:, s, :],
                                         cs["n1_b"][:])

            qkv_sb = act.tile([P, NSUB, 3, D], bf16, tag="qkv_sb")
            for s in range(NSUB):
                y1T = act.tile([P, 3, P], bf16, tag="y1T")
                transp3(lambda j: y1[:, s, j * P:(j + 1) * P],
                        _fancy(y1T[:, 0, 0:1], [[1, 3 * P]]), copy_eng="vec")
                for part in range(3):
                    qp = ps.tile([P, 512], f32, tag="mm", name="qp")
                    for k in range(3):
                        nc.tensor.matmul(qp[:, 0:D], y1T[:, k, :],
                                         cs["wsa"][:, k, part * D:(part + 1) * D],
                                         start=(k == 0), stop=False)
                    nc.tensor.matmul(qp[:, 0:D], cs["ones1"][:, :],
                                     cs["bqkv"][:, part * D:(part + 1) * D],
                                     start=False, stop=True)
                    nc.scalar.activation(out=qkv_sb[:, s, part, :],
                                         in_=qp[:, 0:D], func=AF.Identity)

            for s in range(NSUB):
                # scores
                t1 = act.tile([P, L, H, DH], bf16, tag="t1")
                for half in range(2):
                    kx = ps.tile([P, 2, 512], f32, tag="attn", bufs=2,
                                 name="kx")
                    for i in range(2):
                        nc.tensor.matmul(kx[:, i, 0:D],
                                         cs["bl"][:, 2 * half + i, :],
                                         qkv_sb[:, s, 1, :], start=True,
                                         stop=True)
                    q_b = _fancy(qkv_sb[:, s, 0, 0:1],
                                 [[0, 2], [DH, H], [1, DH]])
                    kxv = _fancy(kx[:, 0, 0:1], [[512, 2], [DH, H], [1, DH]])
                    nc.vector.tensor_tensor(
                        out=t1[:, 2 * half:2 * half + 2, :, :],
                        in0=q_b, in1=kxv, op=AL.mult)
                s_f = stat.tile([P, L, H], f32, tag="s_f")
                nc.vector.reduce_sum(out=s_f[:], in_=t1[:], axis=AX.X)
                # exp via Taylor-3 (Pool-legal plain tensor_tensor ops)
                bLH = lambda t: _fancy(t[:, 0:1], [[0, L], [0, H]])
                sq = stat.tile([P, L, H], f32, tag="sq")
                nc.gpsimd.tensor_tensor(out=sq[:], in0=s_f[:], in1=s_f[:],
                                        op=AL.mult)
                cu = stat.tile([P, L, H], f32, tag="cu")
                nc.gpsimd.tensor_tensor(out=cu[:], in0=sq[:], in1=s_f[:],
                                        op=AL.mult)
                nc.gpsimd.tensor_tensor(out=sq[:], in0=sq[:], in1=bLH(half_t),
                                        op=AL.mult)
                nc.gpsimd.tensor_tensor(out=cu[:], in0=cu[:], in1=bLH(sixth_t),
                                        op=AL.mult)
                e_t = stat.tile([P, L, H], f32, tag="e_t")
                nc.gpsimd.tensor_tensor(out=e_t[:], in0=s_f[:], in1=bLH(one_t),
                                        op=AL.add)
                nc.gpsimd.tensor_tensor(out=e_t[:], in0=e_t[:], in1=sq[:],
                                        op=AL.add)
                nc.gpsimd.tensor_tensor(out=e_t[:], in0=e_t[:], in1=cu[:],
                                        op=AL.add)
                z_t = stat.tile([P, H], f32, tag="z_t")
                nc.vector.reduce_sum(out=z_t[:],
                                     in_=_fancy(e_t[:, 0, 0:1], [[1, H], [H, L]]),
                                     axis=AX.X)
                nc.vector.reciprocal(out=z_t[:], in_=z_t[:])
                a_t = stat.tile([P, L, H], bf16, tag="a_t")
                nc.gpsimd.tensor_tensor(out=a_t[:], in0=e_t[:],
                                        in1=_fancy(z_t[:, 0:1], [[0, L], [1, H]]),
                                        op=AL.mult)

                t2 = act.tile([P, L, H, DH], bf16, tag="t2")
                for half in range(2):
                    vx = ps.tile([P, 2, 512], f32, tag="attn", bufs=2,
                                 name="vx")
                    for i in range(2):
                        nc.tensor.matmul(vx[:, i, 0:D],
                                         cs["bl"][:, 2 * half + i, :],
                                         qkv_sb[:, s, 2, :], start=True,
                                         stop=True)
                    a_b = _fancy(a_t[:, 0, 0:1],
                                 [[H, 2], [1, H], [0, DH]], 2 * half * H)
                    vxv = _fancy(vx[:, 0, 0:1], [[512, 2], [DH, H], [1, DH]])
                    nc.vector.tensor_tensor(
                        out=t2[:, 2 * half:2 * half + 2, :, :],
                        in0=a_b, in1=vxv, op=AL.mult)
                o_sb = act.tile([P, D], bf16, tag="o_sb")
                u01 = act.tile([P, D], bf16, tag="u01")
                nc.gpsimd.tensor_add(u01[:], t2[:, 0, :, :], t2[:, 1, :, :])
                u23 = act.tile([P, D], bf16, tag="u23")
                nc.gpsimd.tensor_add(u23[:], t2[:, 2, :, :], t2[:, 3, :, :])
                nc.gpsimd.tensor_add(o_sb[:], u01[:], u23[:])

                oT = act.tile([P, 3, P], bf16, tag="oT")
                transp3(lambda j: o_sb[:, j * P:(j + 1) * P],
                        _fancy(oT[:, 0, 0:1], [[1, 3 * P]]), copy_eng="act")
                h2ps = ps.tile([P, D], f32, tag="mm", name="h2ps")
                for k in range(3):
                    nc.tensor.matmul(h2ps[:, :], oT[:, k, :], cs["wso"][:, k, :],
                                     start=(k == 0), stop=False)
                nc.tensor.matmul(h2ps[:, :], cs["ones1"][:, :], cs["bso"][:, :],
                                 start=False, stop=True)
                nc.vector.tensor_add(r2[:, s, :], h2ps[:, :], y1[:, s, :])
                rstd2, nmr2 = ln_sub(r2[:, s, :], "ln2")
                nc.scalar.activation(out=y2[:, s, :], in_=r2[:, s, :],
                                     func=AF.Identity,
                                     bias=nmr2[:, 0:1], scale=rstd2[:, 0:1])
                if aff2:
                    nc.gpsimd.tensor_tensor(out=y2[:, s, :], in0=y2[:, s, :],
                                            in1=cs["n2_g"][:], op=AL.mult)
                    nc.gpsimd.tensor_add(y2[:, s, :], y2[:, s, :],
                                         cs["n2_b"][:])

            # deferred gate of previous macro (tanh adjacent to gelu block)
            if pending is not None:
                outf = emit_gate(pending[0])
                nc.sync.dma_start(
                    out=out_d[pending[1] * P:(pending[1] + 1) * P, :],
                    in_=outf[:])
                pending = None

            # ---------- FFN (macro level, fp8 DoubleRow) ----------
            y2T = act.tile([P, 4, 4 * P], f8, tag="y2T")
            nc.gpsimd.memset(y2T[:, 3, :], 0.0)
            for s in range(NSUB):
                transp3(lambda j: y2[:, s, j * P:(j + 1) * P],
                        _fancy(y2T[:, 0, 0:1], [[4 * P, 3], [1, P]], s * P),
                        copy_eng=("act" if s % 2 == 0 else "vec"))
            gl = act.tile([P, 12, 4 * P], f8, tag="gl")
            for ch in range(12):
                f1 = ps.tile([P, 4 * P], f32, tag="mm", name="f1")
                for p in range(2):
                    lhsT = _fancy(cs["w1dr"][:, p, 0, 0:1],
                                  [[4 * D, 2], [1, P]], ch * P)
                    rhs = _fancy(y2T[:, 2 * p, 0:1], [[4 * P, 2], [1, 4 * P]])
                    nc.tensor.matmul(f1[:, :], lhsT, rhs, start=(p == 0),
                                     stop=(p == 1), perf_mode=DR)
                nc.scalar.activation(out=gl[:, ch, :], in_=f1[:, :], func=GELU,
                                     bias=cs["b1col"][:, ch:ch + 1], scale=1.0)
            r3 = act.tile([P, NSUB, D], bf16, tag="r3")
            poolps = ps.tile([P, D], f32, tag="acc", bufs=1, name="poolps")
            for s in range(NSUB):
                f2 = ps.tile([P, D], f32, tag="mm", name="f2")
                for p in range(6):
                    lhsT = _fancy(gl[:, 2 * p, 0:1], [[4 * P, 2], [1, P]],
                                  s * P)
                    rhs = _fancy(cs["w2dr"][:, p, 0, 0:1], [[D, 2], [1, D]])
                    nc.tensor.matmul(f2[:, :], lhsT, rhs, start=(p == 0),
                                     stop=False, perf_mode=DR)
                nc.tensor.matmul(f2[:, :], cs["ones1"][:, :], cs["b2row"][:, :],
                                 start=False, stop=True)
                nc.vector.tensor_add(r3[:, s, :], f2[:, :], y2[:, s, :])
                rstd3, nmr3 = ln_sub(r3[:, s, :], "ln3")
                y3 = act.tile([P, D], bf16, tag="y3")
                nc.vector.tensor_scalar(out=y3[:], in0=r3[:, s, :],
                                        scalar1=rstd3[:, 0:1],
                                        scalar2=nmr3[:, 0:1],
                                        op0=AL.mult, op1=AL.add)
                nc.tensor.matmul(poolps[32 * s:32 * (s + 1), :], cs["pool"][:, :],
                                 y3[:], start=True, stop=True,
                                 tile_position=(0, 32 * s))
            pooled = act.tile([P, D], bf16, tag="pooled")
            nc.scalar.activation(out=pooled[:], in_=poolps[:, :],
                                 func=AF.Identity)
            pending = (pooled, m)

        outf = emit_gate(pending[0])
        nc.sync.dma_start(out=out_d[pending[1] * P:(pending[1] + 1) * P, :],
                          in_=outf[:])

    nc.finalize()
    return nc


_prog = None
_prog_key = None


def kernel(**inputs):
    global _prog, _prog_key
    inputs = {k: np.asarray(v, dtype=np.float32) for k, v in inputs.items()}
    consts = _host_consts(inputs)
    aff1 = not (np.allclose(inputs["n1_g"], 1.0) and np.allclose(inputs["n1_b"], 0.0))
    aff2 = not (np.allclose(inputs["n2_g"], 1.0) and np.allclose(inputs["n2_b"], 0.0))
    key = (aff1, aff2)
    if _prog is None or _prog_key != key:
        _prog = build_program(aff1=aff1, aff2=aff2)
        _prog_key = key
    x = inputs["x"]
    in_maps = []
    for c in range(NCORES):
        m = {"x": np.ascontiguousarray(x[c * BC:(c + 1) * BC])}
        m.update({k: consts[k] for k in CONSTS_META})
        in_maps.append(m)
    res = run_bass_kernel_spmd(_prog, in_maps, core_ids=list(range(NCORES)))
    return np.concatenate([res.results[c]["out"] for c in range(NCORES)], axis=0)


if __name__ == "__main__":
    build_program()
    print("build OK")
